# revision 1
# baseline (speedup 1.0000x reference)
"""Trainium2 Bass kernel for CustomMultiheadAttention (linear attention with
low-rank QKV projections).

Math (fp32 reference):
    q = elu(query @ q_down_w.T @ q_up_w.T + q_up_b) + 1     # feature map
    k = elu(key   @ k_down_w.T @ k_up_w.T + k_up_b) + 1
    v =      value @ v_down_w.T @ v_up_w.T + v_up_b
    per head h (16 heads, head_dim 64):
        kv_h    = k_h^T v_h                  # [64, 64]
        ksum_h  = sum_t k_h[t]               # [64]
        num     = q_h kv_h                   # [S, 64]
        denom   = q_h . ksum_h               # [S]
        attn_h  = num / (denom + 1e-6)
    out = concat_h(attn_h) @ out_w.T + out_b

Sharding: 8 cores = 4 batches x 2 head-groups (8 heads / 512 embed dims per
core). The down projections (full rank 512) are replicated across the 2
head-group cores of a batch; up projections are column-sharded by head group;
the output projection is row-sharded, producing partial sums that the host
adds (plus bias).

Device layouts (per core; all matmul operands are float32r so the PE runs at
full rate with a moving dim of 512):
    xq/xk/xv  [E=1024, S=4096]   inputs pre-transposed on host (token-minor)
    down:  dT[r, t]    weights stationary, x chunks moving
    up(k/v): feat[t, j] (token-major) for the kv/ksum contractions over t
    up(q):   qT[j, t]  (head-dim major) for the num/denom contractions over d
    kv accumulated in SBUF over 8 token-chunks of 512
    out projection -> out_part [4096, 1024], host adds the two partials
"""

import numpy as np

import concourse.bass as bass  # noqa: F401
import concourse.mybir as mybir
import concourse.tile as tile
from concourse import bacc
from concourse.bass_utils import run_bass_kernel_spmd

F32 = mybir.dt.float32
F32R = mybir.dt.float32r
AF = mybir.ActivationFunctionType
OP = mybir.AluOpType

P = 128          # partitions
E = 1024         # embed dim
R = 512          # low rank
G = 512          # head-group width (8 heads x 64)
S = 4096         # sequence length
B = 4            # batch
TC = 512         # token chunk
NCHUNK = S // TC  # 8
NE = E // P      # 8 e-tiles
NR = R // P      # 4 r-tiles
NJ = G // P      # 4 j-tiles
NTS = TC // P    # 4 token subtiles per chunk

_CACHE = {}


def _emit_down(nc, x_dram, wd_sb, dT, ci, xpool, pspool, xtag):
    """dT[r, t] (4 r-tiles) for token chunk ci of one input tensor.

    r-outer: one psum bank accumulates 8 e-tile matmuls per r-tile, so the
    eviction of r-tile rt overlaps the accumulation of rt+1.
    """
    xt = xpool.tile([P, NE, TC], F32R, tag=xtag, name=xtag)
    nc.sync.dma_start(
        out=xt[:],
        in_=x_dram[:, ci * TC:(ci + 1) * TC].rearrange("(a p) t -> p a t", p=P),
    )
    for rt in range(NR):
        ps = pspool.tile([P, TC], F32, tag="dps", name="dps")
        for et in range(NE):
            nc.tensor.matmul(
                ps[:], wd_sb[:, et, P * rt:P * (rt + 1)], xt[:, et, :],
                start=(et == 0), stop=(et == NE - 1),
            )
        nc.vector.tensor_copy(dT[:, rt, :], ps[:])


def _build():
    nc = bacc.Bacc(None, target_bir_lowering=False)

    dp = nc.declare_dram_parameter
    xq = dp("xq", [E, S], F32R, isOutput=False)
    xk = dp("xk", [E, S], F32R, isOutput=False)
    xv = dp("xv", [E, S], F32R, isOutput=False)
    wdq = dp("wdq", [E, R], F32R, isOutput=False)
    wdk = dp("wdk", [E, R], F32R, isOutput=False)
    wdv = dp("wdv", [E, R], F32R, isOutput=False)
    wuq = dp("wuq", [R, G], F32R, isOutput=False)
    wuk = dp("wuk", [R, G], F32R, isOutput=False)
    wuv = dp("wuv", [R, G], F32R, isOutput=False)
    wo = dp("wo", [G, E], F32R, isOutput=False)
    bqt = dp("bqt", [P, NJ], F32, isOutput=False)       # q bias, [128,4] tiles
    bkb = dp("bkb", [P, G], F32, isOutput=False)        # k bias bcast
    bvb = dp("bvb", [P, G], F32, isOutput=False)        # v bias bcast
    rtm = dp("rtm", [P, NJ * 8], F32, isOutput=False)   # R^T head mask tiles
    r8m = dp("r8m", [8, G], F32R, isOutput=False)       # head-replication mask
    onem = dp("onem", [P, 2], F32R, isOutput=False)     # two columns of ones
    out_t = dp("out", [S, E], F32, isOutput=True)

    with tile.TileContext(nc) as tcx:
        from contextlib import ExitStack

        with ExitStack() as root:
            cpool = root.enter_context(tcx.tile_pool(name="consts", bufs=1))
            ones_sb = cpool.tile([P, 2], F32R)
            rt_sb = cpool.tile([P, NJ * 8], F32)
            r8_sb = cpool.tile([8, G], F32R)
            bqt_sb = cpool.tile([P, NJ], F32)
            bkb_sb = cpool.tile([P, G], F32)
            bvb_sb = cpool.tile([P, G], F32)
            kv_acc = cpool.tile([P, NJ, G], F32R)
            kv2 = cpool.tile([P, NJ, P], F32R)   # per-jt block-diag kv pairs
            ksumT = cpool.tile([P, NJ], F32)
            kblk = cpool.tile([P, NJ * 8], F32R)
            wdq_sb = cpool.tile([P, NE, R], F32R)
            nc.sync.dma_start(
                out=wdq_sb[:], in_=wdq.rearrange("(a p) r -> p a r", p=P))
            nc.sync.dma_start(out=ones_sb[:], in_=onem[:])
            nc.sync.dma_start(out=rt_sb[:], in_=rtm[:])
            nc.sync.dma_start(out=r8_sb[:], in_=r8m[:])
            nc.sync.dma_start(out=bqt_sb[:], in_=bqt[:])
            nc.sync.dma_start(out=bkb_sb[:], in_=bkb[:])
            nc.sync.dma_start(out=bvb_sb[:], in_=bvb[:])

            # ---------------- Phase KV ----------------
            with ExitStack() as ph:
                wpool = ph.enter_context(tcx.tile_pool(name="wkv", bufs=1))
                wdk_sb = wpool.tile([P, NE, R], F32R)
                wdv_sb = wpool.tile([P, NE, R], F32R)
                wuk_sb = wpool.tile([P, NR, G], F32R)
                wuv_sb = wpool.tile([P, NR, G], F32R)
                nc.sync.dma_start(
                    out=wdk_sb[:], in_=wdk.rearrange("(a p) r -> p a r", p=P))
                nc.sync.dma_start(
                    out=wdv_sb[:], in_=wdv.rearrange("(a p) r -> p a r", p=P))
                nc.sync.dma_start(
                    out=wuk_sb[:], in_=wuk.rearrange("(a p) j -> p a j", p=P))
                nc.sync.dma_start(
                    out=wuv_sb[:], in_=wuv.rearrange("(a p) j -> p a j", p=P))

                xpool = ph.enter_context(tcx.tile_pool(name="xkv", bufs=3))
                dpool = ph.enter_context(tcx.tile_pool(name="dkv", bufs=1))
                fpool = ph.enter_context(tcx.tile_pool(name="fkv", bufs=2))
                tpool = ph.enter_context(tcx.tile_pool(name="tkv", bufs=2))
                psd = ph.enter_context(
                    tcx.tile_pool(name="psd", bufs=2, space="PSUM"))
                psu = ph.enter_context(
                    tcx.tile_pool(name="psu", bufs=2, space="PSUM"))
                psk = ph.enter_context(
                    tcx.tile_pool(name="psk", bufs=2, space="PSUM"))

                for ci in range(NCHUNK):
                    dTk = dpool.tile([P, NR, TC], F32R, tag="dTk", name="dTk")
                    dTv = dpool.tile([P, NR, TC], F32R, tag="dTv", name="dTv")
                    kfeat = fpool.tile([P, NTS, G], F32R, tag="kfeat",
                                       name="kfeat")
                    vch = fpool.tile([P, NTS, G], F32R, tag="vch", name="vch")

                    _emit_down(nc, xk, wdk_sb, dTk, ci, xpool, psd, "x")
                    # up_k + elu feature map: feat = exp(min(u,0)) + max(u,0)
                    for ts in range(NTS):
                        pu = psu.tile([P, G], F32, tag="pu", name="pu")
                        for rt in range(NR):
                            nc.tensor.matmul(
                                pu[:], dTk[:, rt, P * ts:P * (ts + 1)],
                                wuk_sb[:, rt, :],
                                start=(rt == 0), stop=(rt == NR - 1),
                            )
                        u = tpool.tile([P, G], F32, tag="u", name="u")
                        m = tpool.tile([P, G], F32, tag="m", name="m")
                        ex = tpool.tile([P, G], F32, tag="ex", name="ex")
                        nc.vector.tensor_tensor(u[:], pu[:], bkb_sb[:],
                                                op=OP.add)
                        nc.vector.tensor_scalar_min(m[:], u[:], 0.0)
                        nc.scalar.activation(ex[:], m[:], AF.Exp)
                        nc.vector.scalar_tensor_tensor(
                            kfeat[:, ts, :], u[:], 0.0, ex[:],
                            op0=OP.max, op1=OP.add,
                        )

                    _emit_down(nc, xv, wdv_sb, dTv, ci, xpool, psd, "x")
                    for ts in range(NTS):
                        pu = psu.tile([P, G], F32, tag="pu", name="pu")
                        for rt in range(NR):
                            nc.tensor.matmul(
                                pu[:], dTv[:, rt, P * ts:P * (ts + 1)],
                                wuv_sb[:, rt, :],
                                start=(rt == 0), stop=(rt == NR - 1),
                            )
                        nc.vector.tensor_tensor(
                            vch[:, ts, :], pu[:], bvb_sb[:], op=OP.add)

                    # kv[j1, j2] += sum_t kfeat[t, j1] vch[t, j2]
                    for j1 in range(NJ):
                        pkv = psk.tile([P, G], F32, tag="pkv", name="pkv")
                        for ts in range(NTS):
                            nc.tensor.matmul(
                                pkv[:], kfeat[:, ts, P * j1:P * (j1 + 1)],
                                vch[:, ts, :],
                                start=(ts == 0), stop=(ts == NTS - 1),
                            )
                        if ci == 0:
                            nc.vector.tensor_copy(kv_acc[:, j1, :], pkv[:])
                        else:
                            nc.vector.tensor_tensor(
                                kv_acc[:, j1, :], kv_acc[:, j1, :], pkv[:],
                                op=OP.add)
                    # ksumT[j] += sum_t kfeat[t, j]: kfeat as lhsT, ones rhs
                    for jt in range(NJ):
                        pkt = psk.tile([P, 2], F32, tag="pkt", name="pkt",
                                       bufs=2)
                        for ts in range(NTS):
                            nc.tensor.matmul(
                                pkt[:], kfeat[:, ts, P * jt:P * (jt + 1)],
                                ones_sb[:],
                                start=(ts == 0), stop=(ts == NTS - 1),
                            )
                        if ci == 0:
                            nc.vector.tensor_copy(
                                ksumT[:, jt:jt + 1], pkt[:, 0:1])
                        else:
                            nc.vector.tensor_tensor(
                                ksumT[:, jt:jt + 1], ksumT[:, jt:jt + 1],
                                pkt[:, 0:1], op=OP.add)

                # kblk[j, h] = rt_mask[j, h] * ksumT[j]
                for mt in range(NJ):
                    nc.vector.tensor_scalar(
                        kblk[:, 8 * mt:8 * (mt + 1)],
                        rt_sb[:, 8 * mt:8 * (mt + 1)],
                        ksumT[:, mt:mt + 1], None, op0=OP.mult,
                    )
                # kv2[:, jt, :]: 128x128 block-diag of the jt-tile's 2 heads
                nc.vector.memset(kv2[:].bitcast(F32), 0.0)
                for jt in range(NJ):
                    h0, h1 = 2 * jt, 2 * jt + 1
                    nc.vector.tensor_copy(
                        kv2[0:64, jt, 0:64],
                        kv_acc[0:64, jt, 64 * h0:64 * (h0 + 1)])
                    nc.vector.tensor_copy(
                        kv2[64:P, jt, 64:P],
                        kv_acc[64:P, jt, 64 * h1:64 * (h1 + 1)])

            # ---------------- Phase Q + output ----------------
            with ExitStack() as ph:
                wpool = ph.enter_context(tcx.tile_pool(name="wq", bufs=1))
                wuq_sb = wpool.tile([P, NR, G], F32R)
                wo_sb = wpool.tile([P, NJ, E], F32R)
                nc.sync.dma_start(
                    out=wuq_sb[:], in_=wuq.rearrange("(a p) j -> p a j", p=P))

                xpool = ph.enter_context(tcx.tile_pool(name="xqp", bufs=2))
                dpool = ph.enter_context(tcx.tile_pool(name="dqp", bufs=1))
                qpool = ph.enter_context(tcx.tile_pool(name="qf", bufs=2))
                tpool = ph.enter_context(tcx.tile_pool(name="tq", bufs=2))
                apool = ph.enter_context(tcx.tile_pool(name="attn", bufs=2))
                rpool = ph.enter_context(tcx.tile_pool(name="rep", bufs=2))
                opool = ph.enter_context(tcx.tile_pool(name="osb", bufs=3))
                psd = ph.enter_context(
                    tcx.tile_pool(name="psdq", bufs=2, space="PSUM"))
                psq = ph.enter_context(
                    tcx.tile_pool(name="psq", bufs=2, space="PSUM"))
                psn = ph.enter_context(
                    tcx.tile_pool(name="psn", bufs=1, space="PSUM"))
                psm = ph.enter_context(
                    tcx.tile_pool(name="psm", bufs=2, space="PSUM"))

                def emit_down_q(ci):
                    dTq = dpool.tile([P, NR, TC], F32R, tag="dTq", name="dTq")
                    _emit_down(nc, xq, wdq_sb, dTq, ci, xpool, psd, "x")
                    return dTq

                def emit_up_q(ci, dTq):
                    qT = qpool.tile([P, NJ, TC], F32R, tag="qT", name="qT")
                    for jt in range(NJ):
                        pq = psq.tile([P, TC], F32, tag="pq", name="pq")
                        for rt in range(NR):
                            nc.tensor.matmul(
                                pq[:], wuq_sb[:, rt, P * jt:P * (jt + 1)],
                                dTq[:, rt, :],
                                start=(rt == 0), stop=(rt == NR - 1),
                            )
                        bq_ap = bqt_sb[:, jt:jt + 1]
                        m = tpool.tile([P, TC], F32, tag="qm", name="qm")
                        ex = tpool.tile([P, TC], F32, tag="qe", name="qe")
                        u0 = tpool.tile([P, TC], F32, tag="qu", name="qu")
                        nc.vector.tensor_scalar(
                            m[:], pq[:], bq_ap, 0.0, op0=OP.add, op1=OP.min)
                        nc.scalar.activation(ex[:], m[:], AF.Exp)
                        nc.vector.tensor_scalar(
                            u0[:], pq[:], bq_ap, 0.0, op0=OP.add, op1=OP.max)
                        nc.vector.tensor_tensor(
                            qT[:, jt, :], u0[:], ex[:], op=OP.add)
                    return qT

                def emit_attn_out(ci, qT):
                    attn = apool.tile([P, NJ, TC], F32R, tag="attn",
                                      name="attn")
                    rep = rpool.tile([P, NJ, TC], F32, tag="rep", name="rep")
                    # denom^T[h, t], then num while the reciprocal chain
                    # (DVE) runs, then replicate + divide
                    pdn = psm.tile([8, TC], F32, tag="psm", name="pdn")
                    for jt in range(NJ):
                        nc.tensor.matmul(
                            pdn[:], kblk[:, 8 * jt:8 * (jt + 1)], qT[:, jt, :],
                            start=(jt == 0), stop=(jt == NJ - 1),
                        )
                    dpl = tpool.tile([8, TC], F32, tag="dpl", name="dpl")
                    rcp = tpool.tile([8, TC], F32R, tag="rcp", name="rcp")
                    nc.vector.tensor_scalar_add(dpl[:], pdn[:], 1e-6)
                    with nc.allow_low_precision(
                            reason="f32r is f32-width; rep matmul needs f32r"):
                        nc.vector.reciprocal(rcp[:], dpl[:])
                    pnms = []
                    for jt in range(NJ):
                        pnm = psn.tile([P, TC], F32, tag=f"pnm{jt % 2}", name="pnm")
                        nc.tensor.matmul(
                            pnm[:], kv2[:, jt, :], qT[:, jt, :],
                            start=True, stop=True,
                        )
                        pnms.append(pnm)
                    for jt in range(NJ):
                        prp = psm.tile([P, TC], F32, tag="psm", name="prp")
                        nc.tensor.matmul(
                            prp[:], r8_sb[:, P * jt:P * (jt + 1)], rcp[:],
                            start=True, stop=True,
                        )
                        nc.scalar.copy(rep[:, jt, :], prp[:])
                        nc.vector.tensor_tensor(
                            attn[:, jt, :], pnms[jt][:], rep[:, jt, :],
                            op=OP.mult)

                    # out projection: out[t, o] = sum_j attn[j, t] wo[j, o]
                    for ts in range(NTS):
                        ob = opool.tile([P, 2, TC], F32, tag="ob", name="ob")
                        for oc in range(2):
                            po = psm.tile([P, TC], F32, tag="psm", name="po")
                            for jt in range(NJ):
                                nc.tensor.matmul(
                                    po[:], attn[:, jt, P * ts:P * (ts + 1)],
                                    wo_sb[:, jt, TC * oc:TC * (oc + 1)],
                                    start=(jt == 0), stop=(jt == NJ - 1),
                                )
                            nc.scalar.copy(ob[:, oc, :], po[:])
                        row0 = ci * TC + ts * P
                        nc.sync.dma_start(
                            out=out_t[row0:row0 + P, :].rearrange(
                                "p (a b) -> p a b", a=2),
                            in_=ob[:],
                        )

                nc.sync.dma_start(
                    out=wo_sb[:], in_=wo.rearrange("(a p) o -> p a o", p=P))
                for ci in range(NCHUNK):
                    dT_ci = emit_down_q(ci)
                    qT_ci = emit_up_q(ci, dT_ci)
                    emit_attn_out(ci, qT_ci)

    nc.compile()
    return nc


def _get_nc():
    if "nc" not in _CACHE:
        _CACHE["nc"] = _build()
    return _CACHE["nc"]


def kernel(**inputs):
    query = np.asarray(inputs["query"], dtype=np.float32)
    key = np.asarray(inputs["key"], dtype=np.float32)
    value = np.asarray(inputs["value"], dtype=np.float32)

    # host-side weight prep (tiny); per head-group g
    def prep(g):
        gs = slice(G * g, G * (g + 1))
        d = {}
        d["wdq"] = np.ascontiguousarray(inputs["q_down_w"].T)      # [E, R]
        d["wdk"] = np.ascontiguousarray(inputs["k_down_w"].T)
        d["wdv"] = np.ascontiguousarray(inputs["v_down_w"].T)
        d["wuq"] = np.ascontiguousarray(inputs["q_up_w"][gs].T)    # [R, G]
        d["wuk"] = np.ascontiguousarray(inputs["k_up_w"][gs].T)
        d["wuv"] = np.ascontiguousarray(inputs["v_up_w"][gs].T)
        d["wo"] = np.ascontiguousarray(inputs["out_w"][:, gs].T)   # [G, E]
        d["bqt"] = np.ascontiguousarray(
            inputs["q_up_b"][gs].reshape(NJ, P).T)                 # [128, 4]
        d["bkb"] = np.ascontiguousarray(
            np.broadcast_to(inputs["k_up_b"][gs], (P, G)))
        d["bvb"] = np.ascontiguousarray(
            np.broadcast_to(inputs["v_up_b"][gs], (P, G)))
        return {k2: np.ascontiguousarray(v2, dtype=np.float32)
                for k2, v2 in d.items()}

    wg = [prep(0), prep(1)]

    # head masks / ones
    heads = (np.arange(G) // 64)
    rt_full = (heads[:, None] == np.arange(8)[None, :]).astype(np.float32)
    rtm = np.ascontiguousarray(
        rt_full.reshape(NJ, P, 8).transpose(1, 0, 2).reshape(P, NJ * 8))
    r8m = np.ascontiguousarray(rt_full.T)                          # [8, G]
    onem = np.ones((P, 2), np.float32)

    xT = {}
    for b in range(B):
        xT[("q", b)] = np.ascontiguousarray(query[b].T)
        xT[("k", b)] = np.ascontiguousarray(key[b].T)
        xT[("v", b)] = np.ascontiguousarray(value[b].T)

    in_maps = []
    for c in range(8):
        b, g = divmod(c, 2)
        im = {
            "xq": xT[("q", b)], "xk": xT[("k", b)], "xv": xT[("v", b)],
            "rtm": rtm, "r8m": r8m, "onem": onem,
        }
        im.update(wg[g])
        in_maps.append(im)

    nc = _get_nc()
    # the first execution after a device wedge occasionally dies with
    # NRT_EXEC_UNIT_UNRECOVERABLE; a retry on a clean session recovers
    last_err = None
    for _attempt in range(3):
        try:
            res = run_bass_kernel_spmd(nc, in_maps, core_ids=list(range(8)),
                                       **_CACHE.get("run_kwargs", {}))
            last_err = None
            break
        except Exception as e:  # noqa: BLE001
            last_err = e
            import time
            time.sleep(10)
    if last_err is not None:
        raise last_err
    _CACHE["last_result"] = res

    out_b = np.asarray(inputs["out_b"], dtype=np.float32)
    out = np.empty((B, S, E), np.float32)
    for b in range(B):
        out[b] = res.results[2 * b]["out"] + res.results[2 * b + 1]["out"] \
            + out_b
    return out



# revision 9
# speedup vs baseline: 1.5693x; 1.5693x over previous
"""Trainium2 Bass kernel for CustomMultiheadAttention (linear attention with
low-rank QKV projections).

Math (fp32 reference):
    q = elu(query @ q_down_w.T @ q_up_w.T + q_up_b) + 1     # feature map
    k = elu(key   @ k_down_w.T @ k_up_w.T + k_up_b) + 1
    v =      value @ v_down_w.T @ v_up_w.T + v_up_b
    per head h (16 heads, head_dim 64):
        kv_h    = k_h^T v_h                  # [64, 64]
        ksum_h  = sum_t k_h[t]               # [64]
        num     = q_h kv_h                   # [S, 64]
        denom   = q_h . ksum_h               # [S]
        attn_h  = num / (denom + 1e-6)
    out = concat_h(attn_h) @ out_w.T + out_b

Key optimizations over the straightforward 3-stage pipeline:
  * The down/up projections are fused on the host: W_eff = up @ down is
    [E, E]; column-sharding W_eff by head group gives each core a single
    [E, 512] projection (S*E*G flops) instead of a replicated rank-512
    down stage plus a sharded up stage (1.5x the flops).
  * bf16 data path everywhere (inputs, weights, intermediates); PSUM
    accumulation stays fp32. Halves HBM traffic and SBUF pressure.
  * ksum is fused into the kv matmul via a ones-column appended to v.
  * kv accumulates in PSUM across the whole sequence (one accumulation
    group per head-pair bank spanning all 8 chunks).
  * elu+1 = exp(min(u,0)) + max(u,0); exp/relu on the scalar engine with
    fused per-partition bias, min on DVE, final add on gpsimd.
  * reciprocal of the denominator on the scalar engine (fused +1e-6).

Sharding: 8 cores = 4 batches x 2 head-groups (8 heads / 512 embed dims per
core). The output projection is row-sharded; the host adds the two partial
sums plus bias.

Device layouts (per core):
    xq/xk/xv  [E=1024, S=4096] bf16, inputs pre-transposed on host
    k/v proj: stationary = x-chunk e-tile [128e, 128t], moving = W [e, 512j]
              -> feat [t, j] (token partitions), which the kv contraction
              over t needs
    q proj:   stationary = Wq j-tile [e, 128j], moving = x [e, 512t]
              -> qT [j, t] (head-dim partitions), which num/denom need
    kv:       kfeat pair-tile [t, 128] x [v_pair | ones] [t, 130] -> psum
    out:      stationary = attn [j, 128t], moving = wo [j, 512o] -> [t, o]
"""

import numpy as np
import ml_dtypes

import concourse.bass as bass  # noqa: F401
import concourse.mybir as mybir
import concourse.tile as tile
from concourse import bacc
from concourse.bass_utils import run_bass_kernel_spmd

F32 = mybir.dt.float32
BF16 = mybir.dt.bfloat16
AF = mybir.ActivationFunctionType
OP = mybir.AluOpType

P = 128          # partitions
E = 1024         # embed dim
G = 512          # head-group width (8 heads x 64)
S = 4096         # sequence length
B = 4            # batch
TC = 512         # token chunk
NCHUNK = S // TC  # 8
NE = E // P      # 8 e-tiles
NJ = G // P      # 4 j-tiles
NTS = TC // P    # 4 token subtiles per chunk

_CACHE = {}


def _build():
    nc = bacc.Bacc(None, target_bir_lowering=False)

    dp = nc.declare_dram_parameter
    xq = dp("xq", [E, S], BF16, isOutput=False)
    xk = dp("xk", [E, S], BF16, isOutput=False)
    xv = dp("xv", [E, S], BF16, isOutput=False)
    wq = dp("wq", [E, G], BF16, isOutput=False)
    wk = dp("wk", [E, G], BF16, isOutput=False)
    wv = dp("wv", [E, G], BF16, isOutput=False)
    wo = dp("wo", [G, E], BF16, isOutput=False)
    bqt = dp("bqt", [P, NJ], F32, isOutput=False)       # q bias, [128,4] tiles
    bkb = dp("bkb", [P, G], F32, isOutput=False)        # k bias bcast
    bvb = dp("bvb", [P, G], F32, isOutput=False)        # v bias bcast
    rtm = dp("rtm", [P, NJ * 8], F32, isOutput=False)   # head mask tiles
    r8m = dp("r8m", [8, G], F32, isOutput=False)        # head-replication mask
    out_t = dp("out", [S, E], BF16, isOutput=True)

    with tile.TileContext(nc) as tcx:
        from contextlib import ExitStack

        with ExitStack() as root, nc.allow_low_precision(
                reason="bf16 data path; rel tolerance 2e-2"):
            cpool = root.enter_context(tcx.tile_pool(name="consts", bufs=1))
            # weights needed first come first so their DMAs land first
            wk_sb = cpool.tile([P, NE, G], BF16)
            nc.sync.dma_start(
                out=wk_sb[:], in_=wk.rearrange("(a p) j -> p a j", p=P))
            bkb_sb = cpool.tile([P, G], F32)
            nc.sync.dma_start(out=bkb_sb[:], in_=bkb[:])
            wv_sb = cpool.tile([P, NE, G], BF16)
            nc.sync.dma_start(
                out=wv_sb[:], in_=wv.rearrange("(a p) j -> p a j", p=P))
            bvb_sb = cpool.tile([P, G], F32)
            nc.sync.dma_start(out=bvb_sb[:], in_=bvb[:])
            wq_sb = cpool.tile([P, NE, G], BF16)
            nc.sync.dma_start(
                out=wq_sb[:], in_=wq.rearrange("(a p) j -> p a j", p=P))
            bqt_sb = cpool.tile([P, NJ], F32)
            nc.sync.dma_start(out=bqt_sb[:], in_=bqt[:])
            wo_sb = cpool.tile([P, NJ, E], BF16)
            nc.sync.dma_start(
                out=wo_sb[:], in_=wo.rearrange("(a p) o -> p a o", p=P))
            rt_sb = cpool.tile([P, NJ * 8], F32)
            nc.sync.dma_start(out=rt_sb[:], in_=rtm[:])
            r8_sb = cpool.tile([8, G], F32)
            nc.sync.dma_start(out=r8_sb[:], in_=r8m[:])

            kv2 = cpool.tile([P, NJ, P], BF16)    # block-diag kv head pairs
            kblk = cpool.tile([P, NJ * 8], BF16)  # masked ksum for denom
            kvsb = cpool.tile([P, NJ, 130], F32)  # kv psum staging
            qTall = cpool.tile([P, NCHUNK, NJ, TC], BF16)

            # ---------------- Phase A: k/v/q projections + kv accum --------
            with ExitStack() as ph:
                xpool = ph.enter_context(tcx.tile_pool(name="xa", bufs=2))
                fpool = ph.enter_context(tcx.tile_pool(name="fa", bufs=2))
                tpool = ph.enter_context(tcx.tile_pool(name="ta", bufs=2))
                pskvp = ph.enter_context(
                    tcx.tile_pool(name="pskv", bufs=1, space="PSUM"))
                psp = ph.enter_context(
                    tcx.tile_pool(name="psp", bufs=2, space="PSUM"))
                psq = ph.enter_context(
                    tcx.tile_pool(name="psq", bufs=2, space="PSUM"))
                pskv = [pskvp.tile([P, TC], F32, tag=f"kv{j}", name=f"kv{j}")
                        for j in range(NJ)]

                for ci in range(NCHUNK):
                    # ---- K: proj + elu feature map -> kfeat [t, j] ----
                    xkt = xpool.tile([P, NE, TC], BF16, tag="xk", name="xkt")
                    nc.sync.dma_start(
                        out=xkt[:],
                        in_=xk[:, ci * TC:(ci + 1) * TC].rearrange(
                            "(a p) t -> p a t", p=P))
                    kfeat = fpool.tile([P, NTS, G], BF16, tag="kf",
                                       name="kfeat")
                    for ts in range(NTS):
                        pk = psp.tile([P, G], F32, tag="pp", name="pk")
                        for et in range(NE):
                            nc.tensor.matmul(
                                pk[:], xkt[:, et, P * ts:P * (ts + 1)],
                                wk_sb[:, et, :],
                                start=(et == 0), stop=(et == NE - 1))
                        u = tpool.tile([P, G], F32, tag="u", name="u")
                        nc.vector.tensor_tensor(u[:], pk[:], bkb_sb[:],
                                                op=OP.add)
                        m = tpool.tile([P, G], F32, tag="m", name="m")
                        nc.vector.tensor_scalar_min(m[:], u[:], 0.0)
                        ex = tpool.tile([P, G], F32, tag="ex", name="ex")
                        nc.scalar.activation(ex[:], m[:], AF.Exp)
                        nc.vector.scalar_tensor_tensor(
                            kfeat[:, ts, :], u[:], 0.0, ex[:],
                            op0=OP.max, op1=OP.add)

                    # ---- V: proj + bias -> vch [t, (pair, 128+ones)] ----
                    xvt = xpool.tile([P, NE, TC], BF16, tag="xv", name="xvt")
                    nc.sync.dma_start(
                        out=xvt[:],
                        in_=xv[:, ci * TC:(ci + 1) * TC].rearrange(
                            "(a p) t -> p a t", p=P))
                    vch = fpool.tile([P, NTS, NJ, 132], BF16, tag="vc",
                                     name="vch")
                    nc.gpsimd.memset(vch[:, :, :, P:P + 2], 1.0)
                    for ts in range(NTS):
                        pv = psp.tile([P, G], F32, tag="pp", name="pv")
                        for et in range(NE):
                            nc.tensor.matmul(
                                pv[:], xvt[:, et, P * ts:P * (ts + 1)],
                                wv_sb[:, et, :],
                                start=(et == 0), stop=(et == NE - 1))
                        for j1 in range(NJ):
                            nc.vector.tensor_tensor(
                                vch[:, ts, j1, 0:P],
                                pv[:, P * j1:P * (j1 + 1)],
                                bvb_sb[:, P * j1:P * (j1 + 1)], op=OP.add)

                    # ---- Q: proj + elu -> qTall [j, t] ----
                    xqt = xpool.tile([P, NE, TC], BF16, tag="xq", name="xqt")
                    nc.sync.dma_start(
                        out=xqt[:],
                        in_=xq[:, ci * TC:(ci + 1) * TC].rearrange(
                            "(a p) t -> p a t", p=P))
                    for jt in range(NJ):
                        pq = psq.tile([P, TC], F32, tag="pq", name="pq")
                        for et in range(NE):
                            nc.tensor.matmul(
                                pq[:], wq_sb[:, et, P * jt:P * (jt + 1)],
                                xqt[:, et, :],
                                start=(et == 0), stop=(et == NE - 1))
                        bq_ap = bqt_sb[:, jt:jt + 1]
                        qm = tpool.tile([P, TC], F32, tag="qm", name="qm")
                        nc.vector.tensor_scalar(
                            qm[:], pq[:], bq_ap, 0.0, op0=OP.add, op1=OP.min)
                        qe = tpool.tile([P, TC], F32, tag="qe", name="qe")
                        nc.scalar.activation(qe[:], qm[:], AF.Exp)
                        qu = tpool.tile([P, TC], F32, tag="qu", name="qu")
                        nc.scalar.activation(qu[:], pq[:], AF.Relu,
                                             bias=bq_ap)
                        nc.gpsimd.tensor_tensor(
                            qTall[:, ci, jt, :], qu[:], qe[:], op=OP.add)

                    # ---- KV accum (+ ksum via ones cols) ----
                    for j1 in range(NJ):
                        for ts in range(NTS):
                            nc.tensor.matmul(
                                pskv[j1][:, 0:130],
                                kfeat[:, ts, P * j1:P * (j1 + 1)],
                                vch[:, ts, j1, 0:130],
                                start=(ci == 0 and ts == 0),
                                stop=(ci == NCHUNK - 1 and ts == NTS - 1))

                for j1 in range(NJ):
                    nc.vector.tensor_copy(kvsb[:, j1, :], pskv[j1][:, 0:130])

            # ---- build block-diag kv2 + masked ksum (kblk) ----
            nc.vector.memset(kv2[:], 0.0)
            for jt in range(NJ):
                h0 = 2 * jt
                nc.vector.tensor_copy(kv2[0:64, jt, 0:64],
                                      kvsb[0:64, jt, 0:64])
                nc.vector.tensor_copy(kv2[64:P, jt, 64:P],
                                      kvsb[64:P, jt, 64:P])
                nc.vector.tensor_scalar(
                    kblk[:, 8 * jt:8 * (jt + 1)],
                    rt_sb[:, 8 * jt:8 * (jt + 1)],
                    kvsb[:, jt, 128:129], None, op0=OP.mult)

            # ---------------- Phase B: attention + output projection -------
            with ExitStack() as ph:
                apool = ph.enter_context(tcx.tile_pool(name="ap", bufs=2))
                rpool = ph.enter_context(tcx.tile_pool(name="rp", bufs=2))
                opool = ph.enter_context(tcx.tile_pool(name="op", bufs=3))
                psd = ph.enter_context(
                    tcx.tile_pool(name="psd", bufs=2, space="PSUM"))
                psn = ph.enter_context(
                    tcx.tile_pool(name="psn", bufs=2, space="PSUM"))
                psr = ph.enter_context(
                    tcx.tile_pool(name="psr", bufs=2, space="PSUM"))
                pso = ph.enter_context(
                    tcx.tile_pool(name="pso", bufs=2, space="PSUM"))

                for ci in range(NCHUNK):
                    pdn = psd.tile([8, TC], F32, tag="pd", name="pdn")
                    for jt in range(NJ):
                        nc.tensor.matmul(
                            pdn[:], kblk[:, 8 * jt:8 * (jt + 1)],
                            qTall[:, ci, jt, :],
                            start=(jt == 0), stop=(jt == NJ - 1))
                    rcp = rpool.tile([8, TC], F32, tag="rcp", name="rcp")
                    nc.vector.reciprocal_approx_fast(out=rcp[:], in_=pdn[:])
                    attn = apool.tile([P, NJ, TC], BF16, tag="at",
                                      name="attn")
                    for jt in range(NJ):
                        pnm = psn.tile([P, TC], F32, tag="pn", name="pnm")
                        nc.tensor.matmul(
                            pnm[:], kv2[:, jt, :], qTall[:, ci, jt, :],
                            start=True, stop=True)
                        prp = psr.tile([P, TC], F32, tag="pr", name="prp")
                        nc.tensor.matmul(
                            prp[:], r8_sb[:, P * jt:P * (jt + 1)], rcp[:],
                            start=True, stop=True)
                        rp = rpool.tile([P, TC], F32, tag="rep", name="rep",
                                        bufs=2)
                        nc.scalar.copy(rp[:], prp[:])
                        nc.vector.tensor_tensor(
                            attn[:, jt, :], pnm[:], rp[:], op=OP.mult)

                    for ts in range(NTS):
                        ob = opool.tile([P, 2, TC], BF16, tag="ob", name="ob")
                        for oc in range(2):
                            po = pso.tile([P, TC], F32, tag="po", name="po")
                            for jt in range(NJ):
                                nc.tensor.matmul(
                                    po[:], attn[:, jt, P * ts:P * (ts + 1)],
                                    wo_sb[:, jt, TC * oc:TC * (oc + 1)],
                                    start=(jt == 0), stop=(jt == NJ - 1))
                            if oc == 0:
                                nc.scalar.copy(ob[:, oc, :], po[:])
                            else:
                                nc.vector.tensor_copy(ob[:, oc, :], po[:])
                        row0 = ci * TC + ts * P
                        nc.sync.dma_start(
                            out=out_t[row0:row0 + P, :].rearrange(
                                "p (a b) -> p a b", a=2),
                            in_=ob[:])

    nc.compile()
    return nc


def _get_nc():
    if "nc" not in _CACHE:
        _CACHE["nc"] = _build()
    return _CACHE["nc"]


def _bf16(a):
    return np.ascontiguousarray(np.asarray(a, dtype=np.float32)).astype(
        ml_dtypes.bfloat16)


def kernel(**inputs):
    query = np.asarray(inputs["query"], dtype=np.float32)
    key = np.asarray(inputs["key"], dtype=np.float32)
    value = np.asarray(inputs["value"], dtype=np.float32)

    # host-side fused projection weights (tiny): W_eff = up @ down  [E, E]
    wq_full = np.asarray(inputs["q_up_w"], np.float32) @ np.asarray(
        inputs["q_down_w"], np.float32)
    wk_full = np.asarray(inputs["k_up_w"], np.float32) @ np.asarray(
        inputs["k_down_w"], np.float32)
    wv_full = np.asarray(inputs["v_up_w"], np.float32) @ np.asarray(
        inputs["v_down_w"], np.float32)

    def prep(g):
        gs = slice(G * g, G * (g + 1))
        d = {}
        d["wq"] = _bf16(wq_full[gs].T)                    # [E, G]
        d["wk"] = _bf16(wk_full[gs].T)
        d["wv"] = _bf16(wv_full[gs].T)
        d["wo"] = _bf16(np.asarray(inputs["out_w"], np.float32)[:, gs].T)
        d["bqt"] = np.ascontiguousarray(
            np.asarray(inputs["q_up_b"], np.float32)[gs].reshape(NJ, P).T)
        d["bkb"] = np.ascontiguousarray(np.broadcast_to(
            np.asarray(inputs["k_up_b"], np.float32)[gs], (P, G)))
        d["bvb"] = np.ascontiguousarray(np.broadcast_to(
            np.asarray(inputs["v_up_b"], np.float32)[gs], (P, G)))
        return d

    wg = [prep(0), prep(1)]

    # head masks
    heads = (np.arange(G) // 64)
    rt_full = (heads[:, None] == np.arange(8)[None, :]).astype(np.float32)
    rtm = np.ascontiguousarray(
        rt_full.reshape(NJ, P, 8).transpose(1, 0, 2).reshape(P, NJ * 8))
    r8m = np.ascontiguousarray(rt_full.T)                  # [8, G]

    xT = {}
    for b in range(B):
        xT[("q", b)] = _bf16(query[b].T)
        xT[("k", b)] = _bf16(key[b].T)
        xT[("v", b)] = _bf16(value[b].T)

    in_maps = []
    for c in range(8):
        b, g = divmod(c, 2)
        im = {
            "xq": xT[("q", b)], "xk": xT[("k", b)], "xv": xT[("v", b)],
            "rtm": rtm, "r8m": r8m,
        }
        im.update(wg[g])
        in_maps.append(im)

    nc = _get_nc()
    # the first execution after a device wedge occasionally dies with
    # NRT_EXEC_UNIT_UNRECOVERABLE; a retry on a clean session recovers
    last_err = None
    for _attempt in range(3):
        try:
            res = run_bass_kernel_spmd(nc, in_maps, core_ids=list(range(8)),
                                       **_CACHE.get("run_kwargs", {}))
            last_err = None
            break
        except Exception as e:  # noqa: BLE001
            last_err = e
            import time
            time.sleep(10)
    if last_err is not None:
        raise last_err
    _CACHE["last_result"] = res

    out_b = np.asarray(inputs["out_b"], dtype=np.float32)
    out = np.empty((B, S, E), np.float32)
    for b in range(B):
        out[b] = (res.results[2 * b]["out"].astype(np.float32)
                  + res.results[2 * b + 1]["out"].astype(np.float32)
                  + out_b)
    return out
